# revision 12
# baseline (speedup 1.0000x reference)
"""GPT2 attention (B=2, S=2048, E=1024, H=16, interleaved QKV) on 8 trn2 NeuronCores.

Sharding: core c = 4*b + g handles batch b = c//4 and head group g = c%4
(heads 4g..4g+3): Megatron column-split of W_attn / row-split of W_proj,
data-parallel over batch. Host sums the 4 partial projection outputs per batch.

W_attn columns are host-permuted per core to [q0..q3 | k0..k3 | v0..v3]
(64-wide blocks) so each head's q/k/v share a partition offset, which the
matmul base_partition rule requires.

Per-core dataflow (feature-on-partition layout, fp32r matmuls):
  X [2048,1024] --PE transpose--> X^T
  qkv^T = W_slice^T @ X^T   (features on partitions; q pre-scaled by 1/8)
  per head: S^T[sk,sq] = K^T-stationary matmul (causal lower-triangle blocks only)
            P = exp(S^T + mask)   (masked entries underflow to exact 0)
            PV with a ones-column appended to V -> row 64 = softmax denominators
            normalize via DVE recip + PE ones-broadcast + DVE multiply
  out^T = W_proj_slice^T-stationary matmul over the 256 local channels
"""
import numpy as np

import concourse.bass as bass
import concourse.bacc as bacc
import concourse.tile as tile
from concourse import mybir
from concourse.bass_utils import run_bass_kernel_spmd

F32 = mybir.dt.float32
F32R = mybir.dt.float32r
F16 = mybir.dt.float16

B, S, E, H = 2, 2048, 1024, 16
HD = E // H            # 64
HPC = 4                # heads per core
CW = HPC * 3 * HD      # 768: W_attn cols per core
CP = HPC * HD          # 256: W_proj rows per core
NK = E // 128          # 8 contraction chunks over E
NSQ = S // 512         # 4 sq chunks of 512
NSK = S // 128         # 16 sk chunks of 128

_cache = {}
_last_in_maps = None


def _build():
    from contextlib import ExitStack

    nc = bacc.Bacc("TRN2", target_bir_lowering=False, debug=False, num_devices=8)

    x_d = nc.dram_tensor("x", [16, 128, E], F16, kind="ExternalInput").ap()
    wa_d = nc.dram_tensor("wa", [NK, 128, CW], F16, kind="ExternalInput").ap()
    ba_d = nc.dram_tensor("ba", [6, 128, 1], F32, kind="ExternalInput").ap()
    wp_d = nc.dram_tensor("wp", [2, 128, E], F16, kind="ExternalInput").ap()
    out_d = nc.dram_tensor("out_t", [8, 128, S], F32, kind="ExternalOutput").ap()

    ident16_d = nc.inline_tensor(np.eye(128, dtype=np.float16), name="ident16")
    ones16_d = nc.inline_tensor(np.ones((128, 16, 1), np.float16), name="ones16")
    onesrow_d = nc.inline_tensor(np.ones((128, 64), np.float16), name="onesrow")

    Exp = mybir.ActivationFunctionType.Exp
    Ident = mybir.ActivationFunctionType.Identity

    with tile.TileContext(nc) as tc, ExitStack() as top:
        consts = top.enter_context(tc.tile_pool(name="consts", bufs=1))
        qk_pool = top.enter_context(tc.tile_pool(name="qkvT", bufs=1))
        at_pool = top.enter_context(tc.tile_pool(name="attnT", bufs=1))
        wp_pool = top.enter_context(tc.tile_pool(name="wp", bufs=1))

        # identity first (PE transposes need it), then stream x on the sync
        # queue; weights/consts go on the gpsimd queue so they don't delay x.
        ident16_t = consts.tile([128, 128], F16)
        nc.sync.dma_start(out=ident16_t[:], in_=ident16_d.ap())

        qkvT = [
            qk_pool.tile([128, S], F16, tag=f"qkvT{cc}", name=f"qkvT{cc}")
            for cc in range(6)
        ]
        attnT = [
            at_pool.tile([128, S], F16, tag=f"attnT{c}", name=f"attnT{c}")
            for c in range(2)
        ]

        # ---- phase 1+2: X^T then qkv^T = W^T X^T --------------------------
        with (
            tc.tile_pool(name="xio", bufs=4) as xio,
            tc.tile_pool(name="xTp", bufs=1) as xTp,
            tc.tile_pool(name="wa", bufs=1) as wa_pool,
            tc.tile_pool(name="ps_tr", bufs=4, space="PSUM") as ps_tr,
            tc.tile_pool(name="ps_mm", bufs=4, space="PSUM") as ps_mm,
        ):
            xT = [
                xTp.tile([128, S], F16, tag=f"xT{k}", name=f"xT{k}")
                for k in range(NK)
            ]
            xts = []
            for i in range(16):
                xt = xio.tile([128, E], F16, tag="xt")
                nc.sync.dma_start(out=xt[:], in_=x_d[i])
                xts.append(xt)

            wa_t = wa_pool.tile([128, NK, CW], F16)
            for k in range(NK):
                nc.gpsimd.dma_start(out=wa_t[:, k, :], in_=wa_d[k])
            wp_t = wp_pool.tile([128, 2, E], F16)
            for cc in range(2):
                nc.gpsimd.dma_start(out=wp_t[:, cc, :], in_=wp_d[cc])
            ba_t = consts.tile([128, 6], F32)
            for cc in range(6):
                nc.gpsimd.dma_start(out=ba_t[:, cc : cc + 1], in_=ba_d[cc])
            onesrow_t = consts.tile([128, 64], F16)
            nc.gpsimd.dma_start(out=onesrow_t[:], in_=onesrow_d.ap())
            # additive causal masks for the 4 diagonal-block offsets r:
            # keep (0) where f >= p + 128 r else -1e4  (p=sk in block, f=sq)
            mask_t = consts.tile([128, 4, 512], F32)
            nc.gpsimd.memset(mask_t[:], 0.0)
            for r in range(4):
                nc.gpsimd.affine_select(
                    out=mask_t[:, r, :],
                    in_=mask_t[:, r, :],
                    compare_op=mybir.AluOpType.is_ge,
                    fill=-10000.0,
                    base=-128 * r,
                    pattern=[[1, 512]],
                    channel_multiplier=-1,
                )

            for i in range(16):
                for k in range(NK):
                    ps = ps_tr.tile([128, 128], F16, tag="tr")
                    nc.tensor.transpose(
                        ps[:], xts[i][:, k * 128 : (k + 1) * 128], ident16_t[:]
                    )
                    nc.vector.tensor_copy(
                        xT[k][:, i * 128 : (i + 1) * 128], ps[:]
                    )

            for cc in range(6):
                pss = [
                    ps_mm.tile([128, 512], F32, tag="mm", name="mm_ps")
                    for _ in range(4)
                ]
                for k in range(NK):
                    lhs = wa_t[:, k, cc * 128 : (cc + 1) * 128]
                    for rc in range(4):
                        nc.tensor.matmul(
                            pss[rc][:],
                            lhs,
                            xT[k][:, rc * 512 : (rc + 1) * 512],
                            start=(k == 0),
                            stop=(k == NK - 1),
                        )
                for rc in range(4):
                    nc.scalar.activation(
                        qkvT[cc][:, rc * 512 : (rc + 1) * 512],
                        pss[rc][:],
                        Ident,
                        bias=ba_t[:, cc : cc + 1],
                        scale=0.125 if cc < 2 else 1.0,
                    )

        # ---- phase 3+4: per-head attention --------------------------------
        with (
            tc.tile_pool(name="vb", bufs=1) as vb_pool,
            tc.tile_pool(name="pp", bufs=18) as p_pool,
            tc.tile_pool(name="um", bufs=3) as u_pool,
            tc.tile_pool(name="sm", bufs=3) as small,
            tc.tile_pool(name="ps_tr2", bufs=1, space="PSUM") as ps_tr2,
            tc.tile_pool(name="ps_s", bufs=4, space="PSUM") as ps_s,
            tc.tile_pool(name="ps_bc", bufs=1, space="PSUM") as ps_bc,
            tc.tile_pool(name="ps_pv", bufs=2, space="PSUM") as ps_pv,
        ):
            # all V' tiles upfront so the attention groups never break the
            # PE stream for transposes
            vbs = []
            for h in range(HPC):
                o = (h % 2) * 64
                vT = qkvT[4 + h // 2][o : o + 64, :]
                vb = vb_pool.tile(
                    [128, 16, 65], F16, tag=f"vb{h}", name=f"vb{h}"
                )
                nc.gpsimd.dma_start(
                    out=vb[:, :, 64:65], in_=ones16_d.ap()
                )
                for i in range(NSK):
                    ps = ps_tr2.tile([128, 64], F16, tag="tr2")
                    nc.tensor.transpose(
                        ps[:],
                        vT[:, i * 128 : (i + 1) * 128],
                        ident16_t[o : o + 64, o : o + 64],
                    )
                    nc.vector.tensor_copy(vb[:, i, 0:64], ps[:])
                vbs.append(vb)

            def norm_tail(st):
                pv, rcp_r, h, J = st
                sq = bass.ts(J, 512)
                bc = ps_bc.tile([64, 512], F32, tag="bc", name="bc")
                nc.tensor.matmul(
                    bc[:],
                    onesrow_t[64:65, :],
                    rcp_r[64:65, :],
                    start=True,
                    stop=True,
                )
                u = u_pool.tile([64, 512], F32, tag="u", name="u")
                nc.scalar.copy(u[:], pv[0:64, :])
                if h % 2 == 0:
                    nc.vector.tensor_mul(attnT[h // 2][0:64, sq], u[:], bc[:])
                else:
                    tmp = u_pool.tile(
                        [64, 512], F16, tag="tmpshift", name="tmpshift"
                    )
                    nc.vector.tensor_mul(tmp[:], u[:], bc[:])
                    nc.sync.dma_start(out=attnT[h // 2][64:128, sq], in_=tmp[:])

            def proj_J(J):
                for eo in range(8):
                    ps2 = ps_s.tile([128, 512], F32, tag="s", name="proj_ps")
                    for cc in range(2):
                        nc.tensor.matmul(
                            ps2[:],
                            wp_t[:, cc, eo * 128 : (eo + 1) * 128],
                            attnT[cc][:, J * 512 : (J + 1) * 512],
                            start=(cc == 0),
                            stop=(cc == 1),
                        )
                    ob = u_pool.tile([128, 512], F32, tag="ob", name="ob")
                    nc.vector.tensor_copy(ob[:], ps2[:])
                    nc.sync.dma_start(
                        out=out_d[eo][:, J * 512 : (J + 1) * 512], in_=ob[:]
                    )

            pending = None
            for J in range(NSQ):
                for h in range(HPC):
                    o = (h % 2) * 64
                    qT = qkvT[h // 2][o : o + 64, :]
                    kT = qkvT[2 + h // 2][o : o + 64, :]
                    vb = vbs[h]
                    nblk = 4 * J + 4
                    sq = bass.ts(J, 512)
                    pblks = []
                    for i in range(nblk):
                        sps = ps_s.tile([128, 512], F32, tag="s", name="sps")
                        nc.tensor.matmul(
                            sps[:],
                            kT[:, i * 128 : (i + 1) * 128],
                            qT[:, sq],
                            start=True,
                            stop=True,
                        )
                        r = i - 4 * J
                        if r >= 0:
                            nc.vector.tensor_add(
                                sps[:], sps[:], mask_t[:, r, :]
                            )
                        p = p_pool.tile([128, 512], F16, tag="p", name="p")
                        nc.scalar.activation(p[:], sps[:], Exp)
                        pblks.append(p)
                    # previous group's normalize tail goes here: its inputs
                    # are long since ready, so the PE takes it without a stall
                    if pending is not None:
                        ph, pJ = pending[2], pending[3]
                        norm_tail(pending)
                        pending = None
                        if ph == HPC - 1:
                            proj_J(pJ)
                    pv = ps_pv.tile([65, 512], F32, tag="pv", name="pv")
                    for i, p in enumerate(pblks):
                        nc.tensor.matmul(
                            pv[:],
                            vb[:, i, :],
                            p[:],
                            start=(i == 0),
                            stop=(i == nblk - 1),
                        )
                    # normalize head: DVE/ACT work that runs under the next
                    # group's matmuls. row 64 of pv = softmax denominators.
                    rcp = small.tile([128, 512], F32, tag="rcp", name="rcp")
                    nc.vector.reciprocal(rcp[64:65, :], pv[64:65, :])
                    rcp_r = small.tile([128, 512], F16, tag="rcpr", name="rcpr")
                    nc.scalar.copy(rcp_r[64:65, :], rcp[64:65, :])
                    pending = (pv, rcp_r, h, J)
            norm_tail(pending)
            proj_J(NSQ - 1)

    nc.compile()
    return nc


def _col_perm(g):
    """Per-core W_attn column permutation: [q0..q3 | k0..k3 | v0..v3]."""
    cols = []
    for t in range(3):          # q, k, v
        for h in range(HPC):
            base = (4 * g + h) * 3 * HD + t * HD
            cols.append(np.arange(base, base + HD))
    return np.concatenate(cols)


def kernel(hidden_states, W_attn, b_attn, W_proj, b_proj):
    hidden_states = np.asarray(hidden_states, np.float32)
    W_attn = np.asarray(W_attn, np.float32)
    b_attn = np.asarray(b_attn, np.float32)
    W_proj = np.asarray(W_proj, np.float32)
    b_proj = np.asarray(b_proj, np.float32)

    if "nc" not in _cache:
        _cache["nc"] = _build()
    nc = _cache["nc"]

    # q columns (first 256 of the permuted layout) have scale 1/8 folded into
    # the PSUM->SBUF copy; bias is added after the scale, so pre-scale it.
    bias_scale = np.ones(CW, np.float32)
    bias_scale[: 4 * HD] = 0.125

    in_maps = []
    for c in range(8):
        b, g = divmod(c, 4)
        perm = _col_perm(g)
        wa = np.ascontiguousarray(W_attn[:, perm])
        ba = (b_attn[perm] * bias_scale).astype(np.float32)
        wp = np.ascontiguousarray(W_proj[g * CP : (g + 1) * CP, :])
        in_maps.append(
            {
                "x": np.ascontiguousarray(hidden_states[b]).astype(np.float16).reshape(16, 128, E),
                "wa": wa.astype(np.float16).reshape(NK, 128, CW),
                "ba": ba.reshape(6, 128, 1),
                "wp": wp.astype(np.float16).reshape(2, 128, E),
            }
        )

    global _last_in_maps
    _last_in_maps = in_maps
    res = run_bass_kernel_spmd(nc, in_maps, list(range(8)))

    out = np.zeros((B, S, E), np.float32)
    for c in range(8):
        b = c // 4
        out[b] += res.results[c]["out_t"].reshape(E, S).T
    out += b_proj
    return out


# revision 13
# speedup vs baseline: 1.0887x; 1.0887x over previous
"""GPT2 attention (B=2, S=2048, E=1024, H=16, interleaved QKV) on 8 trn2 NeuronCores.

Sharding: core c = 4*b + g handles batch b = c//4 and head group g = c%4
(heads 4g..4g+3): Megatron column-split of W_attn / row-split of W_proj,
data-parallel over batch. Host sums the 4 partial projection outputs per batch.

W_attn columns are host-permuted per core to [q0..q3 | k0..k3 | v0..v3]
(64-wide blocks) so each head's q/k/v share a partition offset, which the
matmul base_partition rule requires.

Per-core dataflow (feature-on-partition layout, fp32r matmuls):
  X [2048,1024] --PE transpose--> X^T
  qkv^T = W_slice^T @ X^T   (features on partitions; q pre-scaled by 1/8)
  per head: S^T[sk,sq] = K^T-stationary matmul (causal lower-triangle blocks only)
            P = exp(S^T + mask)   (masked entries underflow to exact 0)
            PV with a ones-column appended to V -> row 64 = softmax denominators
            normalize via DVE recip + PE ones-broadcast + DVE multiply
  out^T = W_proj_slice^T-stationary matmul over the 256 local channels
"""
import numpy as np

import concourse.bass as bass
import concourse.bacc as bacc
import concourse.tile as tile
from concourse import mybir
from concourse.bass_utils import run_bass_kernel_spmd

F32 = mybir.dt.float32
F32R = mybir.dt.float32r
F16 = mybir.dt.float16

B, S, E, H = 2, 2048, 1024, 16
HD = E // H            # 64
HPC = 4                # heads per core
CW = HPC * 3 * HD      # 768: W_attn cols per core
CP = HPC * HD          # 256: W_proj rows per core
NK = E // 128          # 8 contraction chunks over E
NSQ = S // 512         # 4 sq chunks of 512
NSK = S // 128         # 16 sk chunks of 128

_cache = {}
_last_in_maps = None


def _build():
    from contextlib import ExitStack

    nc = bacc.Bacc("TRN2", target_bir_lowering=False, debug=False, num_devices=8)

    x_d = nc.dram_tensor("x", [16, 128, E], F16, kind="ExternalInput").ap()
    wa_d = nc.dram_tensor("wa", [NK, 128, CW], F16, kind="ExternalInput").ap()
    ba_d = nc.dram_tensor("ba", [6, 128, 1], F32, kind="ExternalInput").ap()
    wp_d = nc.dram_tensor("wp", [2, 128, E], F16, kind="ExternalInput").ap()
    out_d = nc.dram_tensor("out_t", [8, 128, S], F32, kind="ExternalOutput").ap()

    ident16_d = nc.inline_tensor(np.eye(128, dtype=np.float16), name="ident16")
    ones16_d = nc.inline_tensor(np.ones((128, 16, 1), np.float16), name="ones16")
    onesrow_d = nc.inline_tensor(np.ones((128, 64), np.float16), name="onesrow")

    Exp = mybir.ActivationFunctionType.Exp
    Ident = mybir.ActivationFunctionType.Identity

    with tile.TileContext(nc) as tc, ExitStack() as top:
        consts = top.enter_context(tc.tile_pool(name="consts", bufs=1))
        qk_pool = top.enter_context(tc.tile_pool(name="qkvT", bufs=1))
        at_pool = top.enter_context(tc.tile_pool(name="attnT", bufs=1))
        wp_pool = top.enter_context(tc.tile_pool(name="wp", bufs=1))

        # identity first (PE transposes need it), then stream x on the sync
        # queue; weights/consts go on the gpsimd queue so they don't delay x.
        ident16_t = consts.tile([128, 128], F16)
        nc.sync.dma_start(out=ident16_t[:], in_=ident16_d.ap())

        qkvT = [
            qk_pool.tile([128, S], F16, tag=f"qkvT{cc}", name=f"qkvT{cc}")
            for cc in range(6)
        ]
        attnT = [
            at_pool.tile([128, S], F16, tag=f"attnT{c}", name=f"attnT{c}")
            for c in range(2)
        ]

        # ---- phase 1+2: X^T then qkv^T = W^T X^T --------------------------
        with (
            tc.tile_pool(name="xio", bufs=4) as xio,
            tc.tile_pool(name="xTp", bufs=1) as xTp,
            tc.tile_pool(name="wa", bufs=1) as wa_pool,
            tc.tile_pool(name="ps_tr", bufs=4, space="PSUM") as ps_tr,
            tc.tile_pool(name="ps_mm", bufs=4, space="PSUM") as ps_mm,
        ):
            xT = [
                xTp.tile([128, S], F16, tag=f"xT{k}", name=f"xT{k}")
                for k in range(NK)
            ]
            xts = []
            for i in range(16):
                xt = xio.tile([128, E], F16, tag="xt")
                nc.sync.dma_start(out=xt[:], in_=x_d[i])
                xts.append(xt)

            wa_t = wa_pool.tile([128, NK, CW], F16)
            for k in range(NK):
                nc.gpsimd.dma_start(out=wa_t[:, k, :], in_=wa_d[k])
            wp_t = wp_pool.tile([128, 2, E], F16)
            for cc in range(2):
                nc.gpsimd.dma_start(out=wp_t[:, cc, :], in_=wp_d[cc])
            ba_t = consts.tile([128, 6], F32)
            for cc in range(6):
                nc.gpsimd.dma_start(out=ba_t[:, cc : cc + 1], in_=ba_d[cc])
            onesrow_t = consts.tile([128, 64], F16)
            nc.gpsimd.dma_start(out=onesrow_t[:], in_=onesrow_d.ap())
            # additive causal masks for the 4 diagonal-block offsets r:
            # keep (0) where f >= p + 128 r else -1e4  (p=sk in block, f=sq)
            mask_t = consts.tile([128, 4, 512], F32)
            nc.gpsimd.memset(mask_t[:], 0.0)
            for r in range(4):
                nc.gpsimd.affine_select(
                    out=mask_t[:, r, :],
                    in_=mask_t[:, r, :],
                    compare_op=mybir.AluOpType.is_ge,
                    fill=-10000.0,
                    base=-128 * r,
                    pattern=[[1, 512]],
                    channel_multiplier=-1,
                )

            for i in range(16):
                for k in range(NK):
                    ps = ps_tr.tile([128, 128], F16, tag="tr")
                    nc.tensor.transpose(
                        ps[:], xts[i][:, k * 128 : (k + 1) * 128], ident16_t[:]
                    )
                    nc.vector.tensor_copy(
                        xT[k][:, i * 128 : (i + 1) * 128], ps[:]
                    )

            for cc in range(6):
                pss = [
                    ps_mm.tile([128, 512], F32, tag="mm", name="mm_ps")
                    for _ in range(4)
                ]
                for k in range(NK):
                    lhs = wa_t[:, k, cc * 128 : (cc + 1) * 128]
                    for rc in range(4):
                        nc.tensor.matmul(
                            pss[rc][:],
                            lhs,
                            xT[k][:, rc * 512 : (rc + 1) * 512],
                            start=(k == 0),
                            stop=(k == NK - 1),
                        )
                for rc in range(4):
                    nc.scalar.activation(
                        qkvT[cc][:, rc * 512 : (rc + 1) * 512],
                        pss[rc][:],
                        Ident,
                        bias=ba_t[:, cc : cc + 1],
                        scale=0.125 if cc < 2 else 1.0,
                    )

        # ---- phase 3+4: per-head attention --------------------------------
        with (
            tc.tile_pool(name="vb", bufs=1) as vb_pool,
            tc.tile_pool(name="pp", bufs=18) as p_pool,
            tc.tile_pool(name="um", bufs=3) as u_pool,
            tc.tile_pool(name="sm", bufs=3) as small,
            tc.tile_pool(name="ps_tr2", bufs=2, space="PSUM") as ps_tr2,
            tc.tile_pool(name="ps_s", bufs=3, space="PSUM") as ps_s,
            tc.tile_pool(name="ps_bc", bufs=1, space="PSUM") as ps_bc,
            tc.tile_pool(name="ps_pv", bufs=2, space="PSUM") as ps_pv,
        ):
            # all V' tiles upfront so the attention groups never break the
            # PE stream for transposes
            vbs = []
            for h in range(HPC):
                o = (h % 2) * 64
                vT = qkvT[4 + h // 2][o : o + 64, :]
                vb = vb_pool.tile(
                    [128, 16, 65], F16, tag=f"vb{h}", name=f"vb{h}"
                )
                nc.gpsimd.dma_start(
                    out=vb[:, :, 64:65], in_=ones16_d.ap()
                )
                for i in range(NSK):
                    ps = ps_tr2.tile([128, 64], F16, tag="tr2")
                    nc.tensor.transpose(
                        ps[:],
                        vT[:, i * 128 : (i + 1) * 128],
                        ident16_t[o : o + 64, o : o + 64],
                    )
                    nc.vector.tensor_copy(vb[:, i, 0:64], ps[:])
                vbs.append(vb)

            def norm_tail(st):
                pv, rcp_r, h, J = st
                sq = bass.ts(J, 512)
                bc = ps_bc.tile([64, 512], F32, tag="bc", name="bc")
                nc.tensor.matmul(
                    bc[:],
                    onesrow_t[64:65, :],
                    rcp_r[64:65, :],
                    start=True,
                    stop=True,
                )
                u = u_pool.tile([64, 512], F32, tag="u", name="u")
                nc.scalar.copy(u[:], pv[0:64, :])
                if h % 2 == 0:
                    nc.vector.tensor_mul(attnT[h // 2][0:64, sq], u[:], bc[:])
                else:
                    tmp = u_pool.tile(
                        [64, 512], F16, tag="tmpshift", name="tmpshift"
                    )
                    nc.vector.tensor_mul(tmp[:], u[:], bc[:])
                    nc.sync.dma_start(out=attnT[h // 2][64:128, sq], in_=tmp[:])

            pending = None
            for h in range(HPC):
                o = (h % 2) * 64
                qT = qkvT[h // 2][o : o + 64, :]
                kT = qkvT[2 + h // 2][o : o + 64, :]
                vb = vbs[h]

                for J in range(NSQ):
                    nblk = 4 * J + 4
                    sq = bass.ts(J, 512)
                    pblks = []
                    for i in range(nblk):
                        sps = ps_s.tile([128, 512], F32, tag="s", name="sps")
                        nc.tensor.matmul(
                            sps[:],
                            kT[:, i * 128 : (i + 1) * 128],
                            qT[:, sq],
                            start=True,
                            stop=True,
                        )
                        r = i - 4 * J
                        if r >= 0:
                            nc.vector.tensor_add(
                                sps[:], sps[:], mask_t[:, r, :]
                            )
                        p = p_pool.tile([128, 512], F16, tag="p", name="p")
                        nc.scalar.activation(p[:], sps[:], Exp)
                        pblks.append(p)
                    # previous group's normalize tail goes here: its inputs
                    # are long since ready, so the PE takes it without a stall
                    if pending is not None:
                        norm_tail(pending)
                        pending = None
                    pv = ps_pv.tile([65, 512], F32, tag="pv", name="pv")
                    for i, p in enumerate(pblks):
                        nc.tensor.matmul(
                            pv[:],
                            vb[:, i, :],
                            p[:],
                            start=(i == 0),
                            stop=(i == nblk - 1),
                        )
                    # normalize head: DVE/ACT work that runs under the next
                    # group's matmuls. row 64 of pv = softmax denominators.
                    rcp = small.tile([128, 512], F32, tag="rcp", name="rcp")
                    nc.vector.reciprocal(rcp[64:65, :], pv[64:65, :])
                    rcp_r = small.tile([128, 512], F16, tag="rcpr", name="rcpr")
                    nc.scalar.copy(rcp_r[64:65, :], rcp[64:65, :])
                    pending = (pv, rcp_r, h, J)
            norm_tail(pending)

        # ---- phase 5: projection ------------------------------------------
        with (
            tc.tile_pool(name="ob", bufs=2) as ob_pool,
            tc.tile_pool(name="ps_mm2", bufs=4, space="PSUM") as ps_mm2,
        ):
            for eo in range(8):
                ob = ob_pool.tile([128, S], F32, tag="ob")
                pss = [
                    ps_mm2.tile([128, 512], F32, tag="mm", name="mm_ps")
                    for _ in range(4)
                ]
                for cc in range(2):
                    lhs = wp_t[:, cc, eo * 128 : (eo + 1) * 128]
                    for J in range(4):
                        nc.tensor.matmul(
                            pss[J][:],
                            lhs,
                            attnT[cc][:, J * 512 : (J + 1) * 512],
                            start=(cc == 0),
                            stop=(cc == 1),
                        )
                for J in range(4):
                    nc.vector.tensor_copy(
                        ob[:, J * 512 : (J + 1) * 512], pss[J][:]
                    )
                nc.sync.dma_start(out=out_d[eo], in_=ob[:])

    nc.compile()
    return nc


def _col_perm(g):
    """Per-core W_attn column permutation: [q0..q3 | k0..k3 | v0..v3]."""
    cols = []
    for t in range(3):          # q, k, v
        for h in range(HPC):
            base = (4 * g + h) * 3 * HD + t * HD
            cols.append(np.arange(base, base + HD))
    return np.concatenate(cols)


def kernel(hidden_states, W_attn, b_attn, W_proj, b_proj):
    hidden_states = np.asarray(hidden_states, np.float32)
    W_attn = np.asarray(W_attn, np.float32)
    b_attn = np.asarray(b_attn, np.float32)
    W_proj = np.asarray(W_proj, np.float32)
    b_proj = np.asarray(b_proj, np.float32)

    if "nc" not in _cache:
        _cache["nc"] = _build()
    nc = _cache["nc"]

    # q columns (first 256 of the permuted layout) have scale 1/8 folded into
    # the PSUM->SBUF copy; bias is added after the scale, so pre-scale it.
    bias_scale = np.ones(CW, np.float32)
    bias_scale[: 4 * HD] = 0.125

    in_maps = []
    for c in range(8):
        b, g = divmod(c, 4)
        perm = _col_perm(g)
        wa = np.ascontiguousarray(W_attn[:, perm])
        ba = (b_attn[perm] * bias_scale).astype(np.float32)
        wp = np.ascontiguousarray(W_proj[g * CP : (g + 1) * CP, :])
        in_maps.append(
            {
                "x": np.ascontiguousarray(hidden_states[b]).astype(np.float16).reshape(16, 128, E),
                "wa": wa.astype(np.float16).reshape(NK, 128, CW),
                "ba": ba.reshape(6, 128, 1),
                "wp": wp.astype(np.float16).reshape(2, 128, E),
            }
        )

    global _last_in_maps
    _last_in_maps = in_maps
    res = run_bass_kernel_spmd(nc, in_maps, list(range(8)))

    out = np.zeros((B, S, E), np.float32)
    for c in range(8):
        b = c // 4
        out[b] += res.results[c]["out_t"].reshape(E, S).T
    out += b_proj
    return out


# revision 14
# speedup vs baseline: 1.1212x; 1.0299x over previous
"""GPT2 attention (B=2, S=2048, E=1024, H=16, interleaved QKV) on 8 trn2 NeuronCores.

Sharding: core c = 4*b + g handles batch b = c//4 and head group g = c%4
(heads 4g..4g+3): Megatron column-split of W_attn / row-split of W_proj,
data-parallel over batch. Host sums the 4 partial projection outputs per batch.

W_attn columns are host-permuted per core to [q0..q3 | k0..k3 | v0..v3]
(64-wide blocks) so each head's q/k/v share a partition offset, which the
matmul base_partition rule requires.

Per-core dataflow (feature-on-partition layout, fp32r matmuls):
  X [2048,1024] --PE transpose--> X^T
  qkv^T = W_slice^T @ X^T   (features on partitions; q pre-scaled by 1/8)
  per head: S^T[sk,sq] = K^T-stationary matmul (causal lower-triangle blocks only)
            P = exp(S^T + mask)   (masked entries underflow to exact 0)
            PV with a ones-column appended to V -> row 64 = softmax denominators
            normalize via DVE recip + PE ones-broadcast + DVE multiply
  out^T = W_proj_slice^T-stationary matmul over the 256 local channels
"""
import numpy as np

import concourse.bass as bass
import concourse.bacc as bacc
import concourse.tile as tile
from concourse import mybir
from concourse.bass_utils import run_bass_kernel_spmd

F32 = mybir.dt.float32
F32R = mybir.dt.float32r
F16 = mybir.dt.float16

B, S, E, H = 2, 2048, 1024, 16
HD = E // H            # 64
HPC = 4                # heads per core
CW = HPC * 3 * HD      # 768: W_attn cols per core
CP = HPC * HD          # 256: W_proj rows per core
NK = E // 128          # 8 contraction chunks over E
NSQ = S // 512         # 4 sq chunks of 512
NSK = S // 128         # 16 sk chunks of 128

_cache = {}
_last_in_maps = None


def _build():
    from contextlib import ExitStack

    nc = bacc.Bacc("TRN2", target_bir_lowering=False, debug=False, num_devices=8)

    x_d = nc.dram_tensor("x", [16, 128, E], F16, kind="ExternalInput").ap()
    wa_d = nc.dram_tensor("wa", [NK, 128, CW], F16, kind="ExternalInput").ap()
    ba_d = nc.dram_tensor("ba", [6, 128, 1], F32, kind="ExternalInput").ap()
    wp_d = nc.dram_tensor("wp", [2, 128, E], F16, kind="ExternalInput").ap()
    out_d = nc.dram_tensor("out_t", [8, 128, S], F32, kind="ExternalOutput").ap()

    ident16_d = nc.inline_tensor(np.eye(128, dtype=np.float16), name="ident16")
    ones16_d = nc.inline_tensor(np.ones((128, 16, 1), np.float16), name="ones16")
    onesrow_d = nc.inline_tensor(np.ones((128, 64), np.float16), name="onesrow")

    Exp = mybir.ActivationFunctionType.Exp
    Ident = mybir.ActivationFunctionType.Identity

    with tile.TileContext(nc) as tc, ExitStack() as top:
        consts = top.enter_context(tc.tile_pool(name="consts", bufs=1))
        qk_pool = top.enter_context(tc.tile_pool(name="qkvT", bufs=1))
        at_pool = top.enter_context(tc.tile_pool(name="attnT", bufs=1))
        wp_pool = top.enter_context(tc.tile_pool(name="wp", bufs=1))

        # identity first (PE transposes need it), then stream x on the sync
        # queue; weights/consts go on the gpsimd queue so they don't delay x.
        ident16_t = consts.tile([128, 128], F16)
        nc.sync.dma_start(out=ident16_t[:], in_=ident16_d.ap())

        qkvT = [
            qk_pool.tile([128, S], F16, tag=f"qkvT{cc}", name=f"qkvT{cc}")
            for cc in range(6)
        ]
        attnT = [
            at_pool.tile([128, S], F16, tag=f"attnT{c}", name=f"attnT{c}")
            for c in range(2)
        ]

        # ---- phase 1+2: X^T then qkv^T = W^T X^T --------------------------
        with (
            tc.tile_pool(name="xio", bufs=4) as xio,
            tc.tile_pool(name="xTp", bufs=1) as xTp,
            tc.tile_pool(name="wa", bufs=1) as wa_pool,
            tc.tile_pool(name="ps_tr", bufs=4, space="PSUM") as ps_tr,
            tc.tile_pool(name="ps_mm", bufs=4, space="PSUM") as ps_mm,
        ):
            xT = [
                xTp.tile([128, S], F16, tag=f"xT{k}", name=f"xT{k}")
                for k in range(NK)
            ]
            xts = []
            for i in range(16):
                xt = xio.tile([128, E], F16, tag="xt")
                eng = nc.sync if i % 2 == 0 else nc.scalar
                eng.dma_start(out=xt[:], in_=x_d[i])
                xts.append(xt)

            wa_t = wa_pool.tile([128, NK, CW], F16)
            for k in range(NK):
                nc.gpsimd.dma_start(out=wa_t[:, k, :], in_=wa_d[k])
            wp_t = wp_pool.tile([128, 2, E], F16)
            for cc in range(2):
                nc.gpsimd.dma_start(out=wp_t[:, cc, :], in_=wp_d[cc])
            ba_t = consts.tile([128, 6], F32)
            for cc in range(6):
                nc.gpsimd.dma_start(out=ba_t[:, cc : cc + 1], in_=ba_d[cc])
            onesrow_t = consts.tile([128, 64], F16)
            nc.gpsimd.dma_start(out=onesrow_t[:], in_=onesrow_d.ap())
            # additive causal masks for the 4 diagonal-block offsets r:
            # keep (0) where f >= p + 128 r else -1e4  (p=sk in block, f=sq)
            mask_t = consts.tile([128, 4, 512], F32)
            nc.gpsimd.memset(mask_t[:], 0.0)
            for r in range(4):
                nc.gpsimd.affine_select(
                    out=mask_t[:, r, :],
                    in_=mask_t[:, r, :],
                    compare_op=mybir.AluOpType.is_ge,
                    fill=-10000.0,
                    base=-128 * r,
                    pattern=[[1, 512]],
                    channel_multiplier=-1,
                )

            for i in range(16):
                for k in range(NK):
                    ps = ps_tr.tile([128, 128], F16, tag="tr")
                    nc.tensor.transpose(
                        ps[:], xts[i][:, k * 128 : (k + 1) * 128], ident16_t[:]
                    )
                    nc.vector.tensor_copy(
                        xT[k][:, i * 128 : (i + 1) * 128], ps[:]
                    )

            for cc in range(6):
                pss = [
                    ps_mm.tile([128, 512], F32, tag="mm", name="mm_ps")
                    for _ in range(4)
                ]
                for k in range(NK):
                    lhs = wa_t[:, k, cc * 128 : (cc + 1) * 128]
                    for rc in range(4):
                        nc.tensor.matmul(
                            pss[rc][:],
                            lhs,
                            xT[k][:, rc * 512 : (rc + 1) * 512],
                            start=(k == 0),
                            stop=(k == NK - 1),
                        )
                for rc in range(4):
                    nc.scalar.activation(
                        qkvT[cc][:, rc * 512 : (rc + 1) * 512],
                        pss[rc][:],
                        Ident,
                        bias=ba_t[:, cc : cc + 1],
                        scale=0.125 if cc < 2 else 1.0,
                    )

        # ---- phase 3+4: per-head attention --------------------------------
        with (
            tc.tile_pool(name="vb", bufs=1) as vb_pool,
            tc.tile_pool(name="pp", bufs=18) as p_pool,
            tc.tile_pool(name="um", bufs=3) as u_pool,
            tc.tile_pool(name="sm", bufs=3) as small,
            tc.tile_pool(name="ps_tr2", bufs=2, space="PSUM") as ps_tr2,
            tc.tile_pool(name="ps_s", bufs=3, space="PSUM") as ps_s,
            tc.tile_pool(name="ps_bc", bufs=1, space="PSUM") as ps_bc,
            tc.tile_pool(name="ps_pv", bufs=2, space="PSUM") as ps_pv,
        ):
            # all V' tiles upfront so the attention groups never break the
            # PE stream for transposes
            vbs = []
            for h in range(HPC):
                o = (h % 2) * 64
                vT = qkvT[4 + h // 2][o : o + 64, :]
                vb = vb_pool.tile(
                    [128, 16, 65], F16, tag=f"vb{h}", name=f"vb{h}"
                )
                nc.gpsimd.dma_start(
                    out=vb[:, :, 64:65], in_=ones16_d.ap()
                )
                for i in range(NSK):
                    ps = ps_tr2.tile([128, 64], F16, tag="tr2")
                    nc.tensor.transpose(
                        ps[:],
                        vT[:, i * 128 : (i + 1) * 128],
                        ident16_t[o : o + 64, o : o + 64],
                    )
                    nc.vector.tensor_copy(vb[:, i, 0:64], ps[:])
                vbs.append(vb)

            def norm_tail(st):
                pv, rcp_r, h, J = st
                sq = bass.ts(J, 512)
                bc = ps_bc.tile([64, 512], F32, tag="bc", name="bc")
                nc.tensor.matmul(
                    bc[:],
                    onesrow_t[64:65, :],
                    rcp_r[64:65, :],
                    start=True,
                    stop=True,
                )
                u = u_pool.tile([64, 512], F32, tag="u", name="u")
                nc.scalar.copy(u[:], pv[0:64, :])
                if h % 2 == 0:
                    nc.vector.tensor_mul(attnT[h // 2][0:64, sq], u[:], bc[:])
                else:
                    tmp = u_pool.tile(
                        [64, 512], F16, tag="tmpshift", name="tmpshift"
                    )
                    nc.vector.tensor_mul(tmp[:], u[:], bc[:])
                    nc.sync.dma_start(out=attnT[h // 2][64:128, sq], in_=tmp[:])

            pending = None
            for h in range(HPC):
                o = (h % 2) * 64
                qT = qkvT[h // 2][o : o + 64, :]
                kT = qkvT[2 + h // 2][o : o + 64, :]
                vb = vbs[h]

                for J in range(NSQ):
                    nblk = 4 * J + 4
                    sq = bass.ts(J, 512)
                    pblks = []
                    for i in range(nblk):
                        sps = ps_s.tile([128, 512], F32, tag="s", name="sps")
                        nc.tensor.matmul(
                            sps[:],
                            kT[:, i * 128 : (i + 1) * 128],
                            qT[:, sq],
                            start=True,
                            stop=True,
                        )
                        r = i - 4 * J
                        if r >= 0:
                            nc.vector.tensor_add(
                                sps[:], sps[:], mask_t[:, r, :]
                            )
                        p = p_pool.tile([128, 512], F16, tag="p", name="p")
                        nc.scalar.activation(p[:], sps[:], Exp)
                        pblks.append(p)
                    # previous group's normalize tail goes here: its inputs
                    # are long since ready, so the PE takes it without a stall
                    if pending is not None:
                        norm_tail(pending)
                        pending = None
                    pv = ps_pv.tile([65, 512], F32, tag="pv", name="pv")
                    for i, p in enumerate(pblks):
                        nc.tensor.matmul(
                            pv[:],
                            vb[:, i, :],
                            p[:],
                            start=(i == 0),
                            stop=(i == nblk - 1),
                        )
                    # normalize head: DVE/ACT work that runs under the next
                    # group's matmuls. row 64 of pv = softmax denominators.
                    rcp = small.tile([128, 512], F32, tag="rcp", name="rcp")
                    nc.vector.reciprocal(rcp[64:65, :], pv[64:65, :])
                    rcp_r = small.tile([128, 512], F16, tag="rcpr", name="rcpr")
                    nc.scalar.copy(rcp_r[64:65, :], rcp[64:65, :])
                    pending = (pv, rcp_r, h, J)
            norm_tail(pending)

        # ---- phase 5: projection ------------------------------------------
        with (
            tc.tile_pool(name="ob", bufs=6) as ob_pool,
            tc.tile_pool(name="ps_mm2", bufs=4, space="PSUM") as ps_mm2,
        ):
            for eo in range(8):
                pss = [
                    ps_mm2.tile([128, 512], F32, tag="mm", name="mm_ps")
                    for _ in range(4)
                ]
                for cc in range(2):
                    lhs = wp_t[:, cc, eo * 128 : (eo + 1) * 128]
                    for J in range(4):
                        nc.tensor.matmul(
                            pss[J][:],
                            lhs,
                            attnT[cc][:, J * 512 : (J + 1) * 512],
                            start=(cc == 0),
                            stop=(cc == 1),
                        )
                for J in range(4):
                    ob = ob_pool.tile([128, 512], F32, tag="ob", name="ob")
                    nc.vector.tensor_copy(ob[:], pss[J][:])
                    eng = nc.sync if (eo * 4 + J) % 2 == 0 else nc.scalar
                    eng.dma_start(
                        out=out_d[eo][:, J * 512 : (J + 1) * 512], in_=ob[:]
                    )

    nc.compile()
    return nc


def _col_perm(g):
    """Per-core W_attn column permutation: [q0..q3 | k0..k3 | v0..v3]."""
    cols = []
    for t in range(3):          # q, k, v
        for h in range(HPC):
            base = (4 * g + h) * 3 * HD + t * HD
            cols.append(np.arange(base, base + HD))
    return np.concatenate(cols)


def kernel(hidden_states, W_attn, b_attn, W_proj, b_proj):
    hidden_states = np.asarray(hidden_states, np.float32)
    W_attn = np.asarray(W_attn, np.float32)
    b_attn = np.asarray(b_attn, np.float32)
    W_proj = np.asarray(W_proj, np.float32)
    b_proj = np.asarray(b_proj, np.float32)

    if "nc" not in _cache:
        _cache["nc"] = _build()
    nc = _cache["nc"]

    # q columns (first 256 of the permuted layout) have scale 1/8 folded into
    # the PSUM->SBUF copy; bias is added after the scale, so pre-scale it.
    bias_scale = np.ones(CW, np.float32)
    bias_scale[: 4 * HD] = 0.125

    in_maps = []
    for c in range(8):
        b, g = divmod(c, 4)
        perm = _col_perm(g)
        wa = np.ascontiguousarray(W_attn[:, perm])
        ba = (b_attn[perm] * bias_scale).astype(np.float32)
        wp = np.ascontiguousarray(W_proj[g * CP : (g + 1) * CP, :])
        in_maps.append(
            {
                "x": np.ascontiguousarray(hidden_states[b]).astype(np.float16).reshape(16, 128, E),
                "wa": wa.astype(np.float16).reshape(NK, 128, CW),
                "ba": ba.reshape(6, 128, 1),
                "wp": wp.astype(np.float16).reshape(2, 128, E),
            }
        )

    global _last_in_maps
    _last_in_maps = in_maps
    res = run_bass_kernel_spmd(nc, in_maps, list(range(8)))

    out = np.zeros((B, S, E), np.float32)
    for c in range(8):
        b = c // 4
        out[b] += res.results[c]["out_t"].reshape(E, S).T
    out += b_proj
    return out
